# revision 13
# baseline (speedup 1.0000x reference)
"""Trainium2 Bass kernel for nn_CVXPolicy_Integrator (v3.1).

Computation (per sample):
    h = [t, z]                      # [257]
    p = tanh(h @ W1 + b1) @ W2 + b2 # [256]
    r2 = ||p||^2
    w  = LambertW(r2) via damped fixed point
    ustar = -sqrt(w / r2) * p

Pure data parallel over batch B=131072 across 8 cores (16384 rows each).

Design notes (v2 measured 136us; v3 conservative 124us):
  - scale identity: w*e^w = r2  =>  sqrt(w/r2) = e^{-w/2}.  The whole
    rsqrt bit-trick chain collapses to ONE ACT exp; the output sign is
    folded into negated W2/b2 on the host (p' = -p has the same r2,
    and ustar = scale * p').
  - r2 via bn_stats: one DVE inst covers 2 sample-groups ([128,2,256]
    3D AP, free 512 = HW max) emitting (count, mean, count*var) for
    even/odd element interleaves; r2 = cv_e + cv_o + 128*(m_e^2+m_o^2)
    reconstructed with 5 small chunk-level ops.  ~1.2us/ST on DVE vs
    2.1us for the scalar_tensor_tensor+accum pass.
  - cast-copy PSUM->SBUF as ONE [128,1024] ACT inst per super-tile;
    mm2 writes a single 2-bank PSUM tile.
  - newton: linear init w = A1*r2 + A0, 3 damped fixed-point iters
    w' = a*w + r2*exp(-w + ln(1-a)) with ln(1-a) folded into the ACT
    exp bias.  Chunk = 4 super-tiles so the drain tail is short.
  - DMA: 512KB z loads on sync HWDGE, 1MB stores on gpsimd SWDGE
    (separate rings), const loads on the gpsimd ring so the first z
    load starts immediately.
  - stores leave results in the SBUF-native packed layout
    [128, nst*1024]; the host unpermutes (free).

Layouts:
  zpk[p, st*1024 + h*512 + col] = z[st*512 + col, h*128 + p]   (bf16)
  out[p, st*1024 + k*256 + d]   = ustar[st*512 + k*128 + p, d] (bf16)
"""

import os
import sys

import numpy as np

sys.path.insert(0, "/opt/trn_rl_repo")

# tuning knobs
_USE_BN = os.environ.get("V3_USE_BN", "1") == "1"  # bn_stats r2 (else stt)
_GP_SCALE = int(os.environ.get("V3_GP_SCALE", "0"))  # scale groups on gpsimd

import concourse.bacc as bacc  # noqa: E402
import concourse.bass as bass  # noqa: E402
import concourse.mybir as mybir  # noqa: E402
import concourse.tile as tile  # noqa: E402
from concourse import bass_utils  # noqa: E402

F32 = mybir.dt.float32
BF16 = mybir.dt.bfloat16
AF = mybir.ActivationFunctionType
ALU = mybir.AluOpType

B, D, H = 131072, 256, 100
NCORES = 8
BPC = B // NCORES  # 16384 rows per core
ST = 512  # samples per super-tile
NST = BPC // ST  # 32 super-tiles
CH = 4  # super-tiles per chunk (newton granularity)
LQ = 2  # super-tiles per load DMA (512 KiB)
SQ = 4  # super-tiles per store DMA (1 MiB)
FP_ITERS = 3  # damped fixed-point iterations for LambertW
FP_ALPHA = 0.76  # contraction <=0.27 for r2 in [30, 300]
LN1MA = float(np.log(1.0 - FP_ALPHA))
# linear init fit of LambertW(r2) over r2 in [45, 210] (max err 0.18)
A1 = 0.00670754
A0 = 2.65943288


def build_nc(bpc: int = BPC, compile_bacc: bool = True) -> bass.Bass:
    nst = bpc // ST
    ch = min(CH, nst)
    nch = nst // ch
    lq = min(LQ, nst)
    nlq = nst // lq
    sq = min(SQ, nst)
    wd = 4 * ch  # r2 columns per chunk

    nc = bacc.Bacc("TRN2")

    zpk_d = nc.dram_tensor("zpk", [128, nst * 1024], BF16, kind="ExternalInput")
    tq_d = nc.dram_tensor("tq", [1, bpc], BF16, kind="ExternalInput")
    w1a_d = nc.dram_tensor("w1a", [128, H], BF16, kind="ExternalInput")
    w1b_d = nc.dram_tensor("w1b", [128, H], BF16, kind="ExternalInput")
    w1t_d = nc.dram_tensor("w1t", [1, H], BF16, kind="ExternalInput")
    w2_d = nc.dram_tensor("w2a", [H + 1, D], BF16, kind="ExternalInput")
    b1_d = nc.dram_tensor("b1c", [H, 1], F32, kind="ExternalInput")
    out_d = nc.dram_tensor("out", [128, nst * 1024], BF16, kind="ExternalOutput")

    with tile.TileContext(nc) as tc:
        with (
            tc.tile_pool(name="const", bufs=1) as const,
            tc.tile_pool(name="zp", bufs=6) as zp,
            tc.tile_pool(name="up", bufs=3) as up,
            tc.tile_pool(name="nt", bufs=2) as nt,
            tc.tile_pool(name="aps", bufs=3, space="PSUM") as aps,
            tc.tile_pool(name="pps", bufs=2, space="PSUM") as pps,
        ):
            # consts ride the gpsimd (SWDGE) ring so the sync ring's first
            # z load issues at t=0
            w1a = const.tile([128, H], BF16)
            nc.gpsimd.dma_start(w1a[:], w1a_d[:])
            w1b = const.tile([128, H], BF16)
            nc.gpsimd.dma_start(w1b[:], w1b_d[:])
            w1t = const.tile([1, H], BF16)
            nc.gpsimd.dma_start(w1t[:], w1t_d[:])
            w2a = const.tile([H + 1, D], BF16)
            nc.gpsimd.dma_start(w2a[:], w2_d[:])
            b1c = const.tile([H, 1], F32)
            nc.gpsimd.dma_start(b1c[:], b1_d[:])
            tall = const.tile([1, bpc], BF16)
            nc.gpsimd.dma_start(tall[:], tq_d[:])

            # s tiles: 3 rotating persistent buffers; row H (=100) is the
            # constant-1.0 augmented-bias hidden unit, set once (tanh
            # rewrites rows 0:100 every reuse, so row 100 survives; the
            # memset covers 96:128 because partition starts must be
            # 32-aligned).
            s_tiles = []
            for i in range(3):
                s = const.tile([128, ST], BF16, name=f"s{i}")
                nc.gpsimd.memset(s[96:128, :], 1.0)
                s_tiles.append(s)

            junk = const.tile([128, D], BF16, name="junk")
            # per-partition bias ln(1-a) for the fixed-point exp
            lnb = const.tile([128, 1], F32, name="lnb")
            nc.gpsimd.memset(lnb[:], LN1MA)
            # full-core resident p (bf16, batch-major per 128-group)
            p_sb = const.tile([128, nst * 1024], BF16, name="p_sb")
            # per-chunk bn stats / r2 / scale tiles
            bnst = [
                const.tile([128, ch * 24], F32, name=f"bn_{c}") for c in range(nch)
            ]  # ch*4 groups x 6 stats
            r2t = [const.tile([128, wd], F32, name=f"r2_{c}") for c in range(nch)]
            sct = [const.tile([128, wd], F32, name=f"sc_{c}") for c in range(nch)]

            zq: dict[int, object] = {}
            uq: dict[int, object] = {}

            def emit_load(q: int):
                if not (0 <= q < nlq):
                    return
                zt = zp.tile([128, lq * 1024], BF16, tag="z", name="z_tile")
                nc.sync.dma_start(zt[:], zpk_d[:, q * lq * 1024 : (q + 1) * lq * 1024])
                zq[q] = zt

            def emit_front(st: int):
                """mm1 + tanh for super-tile st (PE front half)."""
                if not (0 <= st < nst):
                    return
                q, qj = st // lq, st % lq
                zt = zq[q]
                c0 = st * ST

                a_ps = aps.tile([128, ST], F32, tag="aps")
                nc.tensor.matmul(
                    a_ps[0:H, :], w1a[:], zt[:, qj * 1024 : qj * 1024 + ST],
                    start=True, stop=False,
                )
                nc.tensor.matmul(
                    a_ps[0:H, :], w1b[:], zt[:, qj * 1024 + ST : (qj + 1) * 1024],
                    start=False, stop=False,
                )
                nc.tensor.matmul(
                    a_ps[0:H, :], w1t[:], tall[0:1, c0 : c0 + ST],
                    start=False, stop=True,
                )

                s = s_tiles[st % 3]
                nc.scalar.activation(s[0:H, :], a_ps[0:H, :], AF.Tanh, bias=b1c[:])

            def emit_back(st: int):
                """mm2 + cast-copy + bn_stats for super-tile st."""
                s = s_tiles[st % 3]
                p_ps = pps.tile([128, 1024], F32, tag="pps")
                for k in range(4):
                    nc.tensor.matmul(
                        p_ps[:, k * D : (k + 1) * D],
                        s[0 : H + 1, k * 128 : (k + 1) * 128],
                        w2a[:],
                        start=True,
                        stop=True,
                    )
                pc0 = st * 1024
                # single ACT cast-copy for the whole super-tile
                nc.scalar.copy(p_sb[:, pc0 : pc0 + 1024], p_ps[:])

                c = st // ch
                if _USE_BN:
                    for k in range(4):
                        jl = (st % ch) * 4 + k
                        nc.vector.bn_stats(
                            bnst[c][:, jl * 6 : (jl + 1) * 6],
                            p_sb[:, pc0 + k * D : pc0 + (k + 1) * D],
                        )
                else:
                    for k in range(4):
                        jl = (st % ch) * 4 + k
                        pk = p_sb[:, pc0 + k * D : pc0 + (k + 1) * D]
                        nc.vector.scalar_tensor_tensor(
                            junk[:],
                            pk,
                            1.0,
                            pk,
                            op0=ALU.mult,
                            op1=ALU.mult,
                            accum_out=r2t[c][:, jl : jl + 1],
                        )

            def newton_steps(c: int):
                """Chunk-c Lambert solve: w' = a*w + r2*exp(-w + ln(1-a)).

                Division-free damped fixed point; the (1-a) factor rides the
                exp bias, the final scale is just exp(-w/2) (sign pre-folded
                into W2/b2 on the host).
                """
                r2 = r2t[c][:]
                tg = f"n{c % 2}"

                w = nt.tile([128, wd], F32, tag=f"{tg}_w", name=f"nt{c % 2}_w")
                ew = nt.tile([128, wd], F32, tag=f"{tg}_ew", name=f"nt{c % 2}_ew")
                t1 = nt.tile([128, wd], F32, tag=f"{tg}_t1", name=f"nt{c % 2}_t1")

                if _USE_BN:
                    # r2 = cv_e + cv_o + 128*(m_e^2 + m_o^2) from bn stats
                    stats = bnst[c][:].rearrange("p (g s) -> p g s", s=6)
                    m_e, cv_e = stats[:, :, 1], stats[:, :, 2]
                    m_o, cv_o = stats[:, :, 4], stats[:, :, 5]
                    t2 = nt.tile([128, wd], F32, tag=f"{tg}_t2", name=f"nt{c % 2}_t2")

                    def combine_a():
                        nc.gpsimd.tensor_mul(t1[:], m_e, m_e)
                        nc.gpsimd.tensor_mul(t2[:], m_o, m_o)

                    def combine_b():
                        nc.gpsimd.tensor_add(t1[:], t1[:], t2[:])
                        nc.gpsimd.tensor_add(t2[:], cv_e, cv_o)
                        nc.gpsimd.tensor_scalar_mul(t1[:], t1[:], 128.0)
                        nc.gpsimd.tensor_add(r2, t1[:], t2[:])

                    yield combine_a
                    yield combine_b

                def init():
                    nc.gpsimd.tensor_scalar(w[:], r2, A1, A0, op0=ALU.mult, op1=ALU.add)

                yield init

                def iter_step():
                    nc.scalar.activation(ew[:], w[:], AF.Exp, scale=-1.0, bias=lnb[:])
                    nc.gpsimd.tensor_mul(t1[:], r2, ew[:])
                    nc.gpsimd.tensor_scalar_mul(w[:], w[:], FP_ALPHA)
                    nc.gpsimd.tensor_add(w[:], w[:], t1[:])

                for _ in range(FP_ITERS):
                    yield iter_step

                def finalize():
                    nc.scalar.activation(sct[c][:], w[:], AF.Exp, scale=-0.5)

                yield finalize

            def emit_phase3(c: int, j: int):
                st = c * ch + j
                q, qj = st // sq, st % sq
                if qj == 0:
                    uq[q] = up.tile([128, sq * 1024], BF16, tag="u", name="u_tile")
                u = uq[q]
                pc0 = st * 1024
                for k in range(4):
                    jl = j * 4 + k
                    eng = nc.gpsimd if k < _GP_SCALE else nc.vector
                    eng.tensor_scalar_mul(
                        u[:, qj * 1024 + k * D : qj * 1024 + (k + 1) * D],
                        p_sb[:, pc0 + k * D : pc0 + (k + 1) * D],
                        sct[c][:, jl : jl + 1],
                    )
                if qj == sq - 1:
                    nc.gpsimd.dma_start(
                        out_d[:, q * sq * 1024 : (q + 1) * sq * 1024], u[:]
                    )

            # flat pipeline over super-tiles:
            #   front(g+1) [mm1+tanh] || back(g) [mm2+copy+bn] || phase3(g-6)
            # newton for a chunk drains fully right after its last back().
            LAG_P3 = 6

            for q in range(4):
                emit_load(q)
            emit_front(0)
            for g in range(nst + LAG_P3):
                if g < nst:
                    if g % lq == 0:
                        emit_load(g // lq + 4)
                    emit_front(g + 1)
                    emit_back(g)
                    if (g + 1) % ch == 0:
                        for step in newton_steps(g // ch):
                            step()
                gp = g - LAG_P3
                if 0 <= gp < nst:
                    emit_phase3(gp // ch, gp % ch)

    if compile_bacc:
        nc.compile()
    return nc


_NC_CACHE: dict[int, bass.Bass] = {}


def _get_nc(bpc: int) -> bass.Bass:
    if bpc not in _NC_CACHE:
        _NC_CACHE[bpc] = build_nc(bpc)
    return _NC_CACHE[bpc]


def make_in_maps(z, t, W1, b1, W2, b2, ncores=NCORES):
    import ml_dtypes

    bf = ml_dtypes.bfloat16
    z = np.ascontiguousarray(z, dtype=np.float32)
    t = np.ascontiguousarray(t, dtype=np.float32)
    W1 = np.asarray(W1, dtype=np.float32)
    b1 = np.asarray(b1, dtype=np.float32)
    W2 = np.asarray(W2, dtype=np.float32)
    b2 = np.asarray(b2, dtype=np.float32)
    bpc = z.shape[0] // ncores
    nst = bpc // ST
    zb = z.astype(bf)
    tb = t.astype(bf)
    w1a = np.ascontiguousarray(W1[1:129].astype(bf))
    w1b = np.ascontiguousarray(W1[129:257].astype(bf))
    w1t = np.ascontiguousarray(W1[0:1].astype(bf))
    # sign folded: p' = -p so ustar = +exp(-w/2) * p'
    w2a = np.ascontiguousarray(
        (-np.concatenate([W2, b2[None, :]], axis=0)).astype(bf)
    )
    b1c = np.ascontiguousarray(b1[:, None])
    in_maps = []
    for c in range(ncores):
        sl = slice(c * bpc, (c + 1) * bpc)
        # zpk[p, st*1024 + h*512 + col] = z[st*512 + col, h*128 + p]
        zpk = np.ascontiguousarray(
            zb[sl].T.reshape(2, 128, nst, ST).transpose(1, 2, 0, 3).reshape(128, nst * 1024)
        )
        tq = np.ascontiguousarray(tb[sl, 0].reshape(1, bpc))
        in_maps.append(
            {
                "zpk": zpk,
                "tq": tq,
                "w1a": w1a,
                "w1b": w1b,
                "w1t": w1t,
                "w2a": w2a,
                "b1c": b1c,
            }
        )
    return in_maps


def _unpack_out(res_out: np.ndarray, bpc: int = BPC) -> np.ndarray:
    # out[p, st*1024 + k*256 + d] = ustar[st*512 + k*128 + p, d]
    nst = bpc // ST
    return (
        res_out.reshape(128, nst, 4, D)
        .transpose(1, 2, 0, 3)
        .reshape(bpc, D)
        .astype(np.float32)
    )


def kernel(z, t, W1, b1, W2, b2):
    in_maps = make_in_maps(z, t, W1, b1, W2, b2)
    nc = _get_nc(BPC)
    res = bass_utils.run_bass_kernel_spmd(nc, in_maps, list(range(NCORES))).results
    return np.concatenate(
        [_unpack_out(np.asarray(res[c]["out"])) for c in range(NCORES)], axis=0
    )
